# revision 18
# baseline (speedup 1.0000x reference)
"""Trainium2 Bass kernel for nn_Attention (topk_masking).

reference:
    h = tanh(x @ W1 + b1); e = h @ W2 + b2            # [B,T,1]
    thr = sort(e, axis=1)[:, T//2]                    # per-sample median-index value
    mask: keep e < thr; softmax over kept; out = sum_t beta_t * x_t  -> [B,D,1,1]

Sharding: B=32 across 8 cores (4 samples/core), fully data-parallel.

Per-core pipeline:
  pass1: hT = tanh(W1^T x^T + b1) via fp32 matmuls (xT streamed from DRAM),
         e = W2^T hT (fp32 matmuls, M=1), e rows bounced through DRAM.
  bisect: batched over 4 samples on an E[128,128] relayout; 35 iterations of
          count(e < mid) vs 2048, then exact theta = min{e >= lo} so the kept
          set matches sort()[2048] bit-exactly.
  softmax: beta = exp(e - theta) * [e < theta] / Z  (masked to -1e8 pre-exp).
  pass2: out[d] = sum_t beta_t x[t,d] on VectorE via tensor_tensor_reduce over
         a bf16 copy of xT (beta broadcast across partitions by GpSimd).

b2 is dropped: it shifts e and thr equally and softmax is shift-invariant.
"""
import os
import sys

sys.path.insert(0, "/opt/trn_rl_repo")

import numpy as np
import ml_dtypes

import concourse.bass as bass  # noqa: F401
from concourse import bacc, bass_isa
import concourse.tile as tile
import concourse.mybir as mybir
from concourse.bass_utils import run_bass_kernel_spmd

F32 = mybir.dt.float32
BF16 = mybir.dt.bfloat16
U8 = mybir.dt.uint8
AF = mybir.ActivationFunctionType
ALU = mybir.AluOpType
AX = mybir.AxisListType

BSH, T, D, H = 4, 4096, 1024, 256
TT = 512  # pass1 T-tile
NEG_BIG = -99999999.0
N_ITER = int(os.environ.get("K_NITER", "40"))
PHASE = int(os.environ.get("K_PHASE", "4"))  # 1=p1, 2=+bisect, 3=+softmax, 4=full


def build():
    nc = bacc.Bacc(trn_type="TRN2", target_bir_lowering=False)

    xT = nc.declare_dram_parameter("xT", [BSH, 128, 8, T], F32, isOutput=False)
    xTb = nc.declare_dram_parameter("xTb", [BSH, 128, 8, T], BF16, isOutput=False)
    w1s = nc.declare_dram_parameter("w1s", [128, 8, H], F32, isOutput=False)
    b1s = nc.declare_dram_parameter("b1s", [128, 2], F32, isOutput=False)
    w2s = nc.declare_dram_parameter("w2s", [128, 2], F32, isOutput=False)
    sel = nc.declare_dram_parameter("sel", [128, BSH], F32, isOutput=False)
    selT = nc.declare_dram_parameter("selT", [BSH, 128], F32, isOutput=False)
    out = nc.declare_dram_parameter("out", [BSH, 8, 128], F32, isOutput=True)

    with tile.TileContext(nc) as tc:
        with tc.tile_pool(name="w", bufs=1) as wpool, \
             tc.tile_pool(name="x", bufs=3) as xpool, \
             tc.tile_pool(name="h", bufs=3) as hpool, \
             tc.tile_pool(name="e", bufs=1) as epool, \
             tc.tile_pool(name="bis", bufs=1) as bpool, \
             tc.tile_pool(name="p2", bufs=3) as p2pool, \
             tc.tile_pool(name="ps", bufs=2, space="PSUM") as pspool, \
             tc.tile_pool(name="pse", bufs=2, space="PSUM") as psepool, \
             tc.tile_pool(name="psb", bufs=1, space="PSUM") as psbpool, \
             tc.tile_pool(name="dram", bufs=1, space="DRAM") as dpool:

            e_dram = dpool.tile([BSH, T], F32, tag="e_dram")
            sel_sb = wpool.tile([128, BSH], F32, tag="sel")
            nc.sync.dma_start(sel_sb[:], sel.ap())
            selT_sb = wpool.tile([BSH, 128], F32, tag="selT")
            nc.sync.dma_start(selT_sb[:], selT.ap())
            w1_sb = wpool.tile([128, 8, H], F32, tag="w1")
            nc.sync.dma_start(w1_sb[:], w1s.ap())
            b1_sb = wpool.tile([128, 2], F32, tag="b1")
            nc.sync.dma_start(b1_sb[:], b1s.ap())
            w2_sb = wpool.tile([128, 2], F32, tag="w2")
            nc.sync.dma_start(w2_sb[:], w2s.ap())

            # ---------------- pass 1 ----------------
            for b in range(BSH):
                for ti in range(T // TT):
                    sl = slice(ti * TT, (ti + 1) * TT)
                    xt = xpool.tile([128, 8, TT], F32, tag="xt")
                    nc.sync.dma_start(xt[:], xT.ap()[b, :, :, sl])
                    hs = []
                    for hh in range(2):
                        ps = pspool.tile([128, TT], F32, tag="hps")
                        for dc in range(8):
                            nc.tensor.matmul(
                                ps[:],
                                w1_sb[:, dc, hh * 128 : (hh + 1) * 128],
                                xt[:, dc, :],
                                start=(dc == 0),
                                stop=(dc == 7),
                            )
                        hsb = hpool.tile([128, TT], F32, tag="h")
                        nc.scalar.activation(
                            hsb[:], ps[:], AF.Tanh, bias=b1_sb[:, hh : hh + 1]
                        )
                        hs.append(hsb)
                    eps = psepool.tile([1, TT], F32, tag="eps")
                    nc.tensor.matmul(eps[:], w2_sb[:, 0:1], hs[0][:], start=True, stop=False)
                    nc.tensor.matmul(eps[:], w2_sb[:, 1:2], hs[1][:], start=False, stop=True)
                    estage = hpool.tile([1, TT], F32, tag="estage")
                    nc.scalar.copy(estage[:], eps[:])
                    nc.sync.dma_start(e_dram[b : b + 1, sl], estage[:])

            thpos = bpool.tile([128, 1], F32, tag="thpos")
            vneg = bpool.tile([128, 1], F32, tag="vneg")
            if PHASE >= 2:
                # --------------- bisection ---------------
                E = bpool.tile([128, 128], F32, tag="E")
                for b in range(BSH):
                    # E[32b+lp, f] = e[b, lp*128 + f]
                    nc.sync.dma_start(
                        E[32 * b : 32 * b + 32, :],
                        e_dram[b].rearrange("(lp f) -> lp f", lp=32),
                    )

                lo = bpool.tile([128, 1], F32, tag="lo")
                hi = bpool.tile([128, 1], F32, tag="hi")
                nc.vector.memset(lo[:], -17.0)
                nc.vector.memset(hi[:], 17.0)
                mid = bpool.tile([128, 1], F32, tag="mid")
                cmp_t = bpool.tile([128, 128], U8, tag="cmp")
                cnt = bpool.tile([128, 1], F32, tag="cnt")
                cntb = bpool.tile([128, 1], F32, tag="cntb")
                msk = bpool.tile([128, 1], U8, tag="msk")

                c4_sb = bpool.tile([BSH, 1], F32, tag="c4sb")
                for _ in range(N_ITER):
                    nc.vector.tensor_scalar(mid[:], lo[:], hi[:], 0.5, ALU.add, ALU.mult)
                    nc.vector.tensor_scalar(
                        cmp_t[:], E[:], mid[:], 0.0, ALU.is_lt, ALU.add, accum_out=cnt[:]
                    )
                    # per-sample totals: contraction over all 128 partitions with a
                    # block-indicator, then broadcast back to per-partition form
                    c4_ps = psbpool.tile([BSH, 1], F32, tag="c4ps")
                    nc.tensor.matmul(c4_ps[:], sel_sb[:], cnt[:], start=True, stop=True)
                    nc.scalar.copy(c4_sb[:], c4_ps[:])
                    cb_ps = psbpool.tile([128, 1], F32, tag="cbps")
                    nc.tensor.matmul(cb_ps[:], selT_sb[:], c4_sb[:], start=True, stop=True)
                    nc.vector.tensor_scalar(msk[:], cb_ps[:], 2048.5, None, ALU.is_lt)
                    nc.vector.copy_predicated(lo[:], msk[:], mid[:])
                    nc.vector.tensor_scalar(msk[:], cb_ps[:], 2048.5, None, ALU.is_ge)
                    nc.vector.copy_predicated(hi[:], msk[:], mid[:])

                # theta = lo: after N_ITER halvings of [-17,17] the interval is
                # ~3e-11 wide, so {e < lo} == {e < sort(e)[2048]} except for
                # astronomically unlikely order-stat near-ties.
                nc.vector.tensor_copy(thpos[:], lo[:])
                nc.vector.tensor_scalar(vneg[:], lo[:], -1.0, None, ALU.mult)

            beta_rows = []
            if PHASE >= 3:
                # ---------- softmax (vectorized over the 4 samples) ----------
                e_all4 = epool.tile([BSH, T], F32, tag="e_all4")
                nc.sync.dma_start(e_all4[:], e_dram[:, :])
                nbig4 = epool.tile([BSH, T], F32, tag="scr4T", name="nbig4")
                nc.vector.memset(nbig4[:], NEG_BIG)
                tp4 = bpool.tile([BSH, 1], F32, tag="tp4")
                tn4 = bpool.tile([BSH, 1], F32, tag="tn4")
                for b in range(BSH):
                    nc.sync.dma_start(tp4[b : b + 1, :], thpos[32 * b : 32 * b + 1, :])
                    nc.sync.dma_start(tn4[b : b + 1, :], vneg[32 * b : 32 * b + 1, :])
                m4 = epool.tile([BSH, T], U8, tag="m4")
                nc.vector.tensor_scalar(m4[:], e_all4[:], tp4[:], None, ALU.is_ge)
                nc.vector.copy_predicated(e_all4[:], m4[:], nbig4[:])
                u4 = epool.tile([BSH, T], F32, tag="scr4T", name="u4")
                z4 = bpool.tile([BSH, 1], F32, tag="z4")
                nc.scalar.activation(
                    u4[:], e_all4[:], AF.Exp, bias=tn4[:], scale=1.0, accum_out=z4[:]
                )
                rz4 = bpool.tile([BSH, 1], F32, tag="rz4")
                nc.vector.reciprocal(rz4[:], z4[:])
                beta4 = epool.tile([BSH, T], BF16, tag="beta4")
                nc.vector.tensor_scalar(beta4[:], u4[:], rz4[:], None, ALU.mult)

            if PHASE >= 4:
                # --------------- pass 2 ---------------
                # out[d] = sum_t beta_t x[t,d]: DVE multiplies xb by the
                # partition-broadcast beta row, ScalarE reduces each dc chunk
                # via activation(Copy, accum_out), DVE accumulates over ti.
                for b in range(BSH):
                    accs = p2pool.tile([128, 8], F32, tag=f"acc{b}", bufs=1,
                                       name=f"accs{b}")
                    nc.vector.memset(accs[:], 0.0)
                    # partition_broadcast sources must live on partition 0
                    brow = epool.tile([1, T], BF16, tag="brow", name=f"brow{b}")
                    nc.sync.dma_start(brow[:], beta4[b : b + 1, :])
                    for ti in range(T // TT):
                        sl = slice(ti * TT, (ti + 1) * TT)
                        ub = p2pool.tile([128, 1, TT], BF16, tag="ub")
                        nc.gpsimd.partition_broadcast(
                            ub[:, 0, :], brow[:, sl], channels=128
                        )
                        xb = p2pool.tile([128, 8, TT], BF16, tag="xb")
                        nc.sync.dma_start(xb[:], xTb.ap()[b, :, :, sl])
                        nc.vector.tensor_tensor(
                            out=xb[:], in0=xb[:],
                            in1=ub[:].broadcast_to([128, 8, TT]), op=ALU.mult,
                        )
                        cur = p2pool.tile([128, 8], F32, tag="cur")
                        junk = p2pool.tile([128, TT], BF16, tag="junk")
                        for dc in range(8):
                            nc.scalar.activation(
                                junk[:], xb[:, dc, :], AF.Copy,
                                accum_out=cur[:, dc : dc + 1],
                            )
                        nc.vector.tensor_tensor(
                            out=accs[:], in0=accs[:], in1=cur[:], op=ALU.add
                        )
                    for dc in range(8):
                        nc.sync.dma_start(out.ap()[b, dc, :], accs[:, dc : dc + 1])
            else:
                zt = p2pool.tile([128, 8], F32, tag="zt")
                nc.vector.memset(zt[:], float(PHASE))
                if PHASE >= 2:
                    nc.vector.tensor_copy(zt[:, 0:1], thpos[:])
                    if os.environ.get("K_DEBUG"):
                        nc.vector.tensor_copy(zt[:, 1:2], E[:, 0:1])
                        nc.vector.tensor_copy(zt[:, 2:3], cnt[:])
                        nc.vector.tensor_copy(zt[:, 4:5], lo[:])
                        nc.vector.tensor_copy(zt[:, 5:6], hi[:])
                        nc.vector.tensor_copy(zt[:, 6:7], mid[:])
                for b in range(BSH):
                    for dc in range(8):
                        nc.sync.dma_start(out.ap()[b, dc, :], zt[:, dc : dc + 1])

    nc.finalize()
    return nc


_NC_CACHE = None


def _get_nc():
    global _NC_CACHE
    if _NC_CACHE is None:
        _NC_CACHE = build()
    return _NC_CACHE


def make_in_maps(x, W1, b1, W2, b2):
    del b2  # shift-invariant: no effect on the output
    x = np.asarray(x, dtype=np.float32)
    W1 = np.asarray(W1, dtype=np.float32)
    b1 = np.asarray(b1, dtype=np.float32).reshape(H)
    W2 = np.asarray(W2, dtype=np.float32).reshape(H)

    w1s = np.ascontiguousarray(W1.reshape(8, 128, H).transpose(1, 0, 2))
    b1s = np.ascontiguousarray(b1.reshape(2, 128).T)
    w2s = np.ascontiguousarray(W2.reshape(2, 128).T)
    sel = np.zeros((128, BSH), dtype=np.float32)
    for b in range(BSH):
        sel[32 * b : 32 * b + 32, b] = 1.0
    selT = np.ascontiguousarray(sel.T)

    in_maps = []
    for c in range(8):
        xs = x[4 * c : 4 * c + 4]  # [4, T, D]
        xt = np.ascontiguousarray(
            xs.transpose(0, 2, 1).reshape(BSH, 8, 128, T).transpose(0, 2, 1, 3)
        )  # [4, 128, 8, T]; xt[b,p,dc,t] = x[b,t,dc*128+p]
        in_maps.append(
            {
                "xT": xt,
                "xTb": xt.astype(ml_dtypes.bfloat16),
                "w1s": w1s,
                "b1s": b1s,
                "w2s": w2s,
                "sel": sel,
                "selT": selT,
            }
        )
    return in_maps


def kernel(x, W1, b1, W2, b2):
    nc = _get_nc()
    in_maps = make_in_maps(x, W1, b1, W2, b2)
    res = run_bass_kernel_spmd(nc, in_maps, core_ids=list(range(8)))
    outs = [res.results[c]["out"].reshape(BSH, 1024) for c in range(8)]
    full = np.concatenate(outs, axis=0).astype(np.float32)  # [32, 1024]
    return full[:, :, None, None]
